# revision 17
# baseline (speedup 1.0000x reference)
"""Multi-head attention (B=4, T=2048, D=1024, H=16) on 8 TRN2 NeuronCores.

Sharding: core c handles batch b = c//2 and head-half hh = c%2 (8 heads,
512 of the 1024 channel dims). Each core computes its half of the head
outputs and a row-sharded output projection, producing a partial
[T, D] output. Host unshard: out[b] = partial[2b] + partial[2b+1]
+ b_o + b_v @ w_o.T (the value-bias contribution commutes through
attention because softmax rows sum to 1).

All matmul operands are bf16 (inputs converted on host); accumulation is
fp32 in PSUM. Scores PSUM is double-buffered ([128,1024] x 2 = 4 banks)
so the Exp on the activation engine overlaps the score/AV matmuls; the
other 4 banks rotate between AV accumulators, out-projection and
interleaved Q-projection tiles. The out-projection of t-block tq-1 and
the Q-projection of tq+1 are interleaved into the attention matmul
stream of tq so the tensor engine never idles long enough for the HAM
clock gate to drop it to 1.2 GHz.
"""

from contextlib import ExitStack

import ml_dtypes
import numpy as np

import concourse.bass as bass
import concourse.mybir as mybir
import concourse.tile as tile
from concourse import bacc
from concourse.bass_utils import run_bass_kernel_spmd

B, T, D = 4, 2048, 1024
H = 16
DH = 64  # head dim
HALF = 512  # channels per core (8 heads)
N_CORES = 8

F32 = mybir.dt.float32
BF16 = mybir.dt.bfloat16

TB = 512  # t-block (moving free dim)
NTB = T // TB  # 4
KB = 128  # contraction block
NKB = D // KB  # 8
NJB = HALF // KB  # 4 j-blocks of the half
NTK = T // KB  # 16 key blocks
CH = 3  # exp-chunk slots (3 x 512 = one 3-bank psum tile)


def build_kernel():
    nc = bacc.Bacc(
        "TRN2", target_bir_lowering=False, debug=False, num_devices=N_CORES
    )
    xqT = nc.dram_tensor("xqT", [D, T], BF16, kind="ExternalInput").ap()
    xkT = nc.dram_tensor("xkT", [D, T], BF16, kind="ExternalInput").ap()
    xvT = nc.dram_tensor("xvT", [D, T], BF16, kind="ExternalInput").ap()
    wqT = nc.dram_tensor("wqT", [D, HALF], BF16, kind="ExternalInput").ap()
    wkT = nc.dram_tensor("wkT", [D, HALF], BF16, kind="ExternalInput").ap()
    wvT = nc.dram_tensor("wvT", [D, HALF], BF16, kind="ExternalInput").ap()
    woT = nc.dram_tensor("woT", [HALF, D], BF16, kind="ExternalInput").ap()
    bq = nc.dram_tensor("bq", [HALF, 1], F32, kind="ExternalInput").ap()
    bk = nc.dram_tensor("bk", [HALF, 1], F32, kind="ExternalInput").ap()
    partial = nc.dram_tensor("partial", [T, D], BF16, kind="ExternalOutput").ap()

    with tile.TileContext(nc) as tc, ExitStack() as ctx:
        p_const = ctx.enter_context(tc.tile_pool(name="const", bufs=1))
        p_x = ctx.enter_context(tc.tile_pool(name="x", bufs=1))
        p_kt = ctx.enter_context(tc.tile_pool(name="kt", bufs=NJB))
        p_qt = ctx.enter_context(tc.tile_pool(name="qt", bufs=NJB))
        p_v = ctx.enter_context(tc.tile_pool(name="v", bufs=NTK))
        p_ex = ctx.enter_context(tc.tile_pool(name="ex", bufs=2))
        p_ot = ctx.enter_context(tc.tile_pool(name="ot", bufs=2 * NJB))
        p_sm = ctx.enter_context(tc.tile_pool(name="sm", bufs=2))
        p_st = ctx.enter_context(tc.tile_pool(name="st", bufs=2))
        # PSUM: scores 2x3 banks + av accumulators 2x1 = 8. Interleaved
        # out-/q-proj units borrow score-pool slots (their WAR hazard is
        # just the slot's previous exp, same as a chunk); the "av" tag is
        # separate so units never rotate into an accumulating AV slot.
        p_sc = ctx.enter_context(tc.tile_pool(name="sc", bufs=2, space="PSUM"))
        p_ap = ctx.enter_context(tc.tile_pool(name="ap", bufs=2, space="PSUM"))

        # ---- weights / biases / staged activations, in first-use order ----
        x_k = p_x.tile([KB, NKB, T], BF16, tag="xk")
        x_v = p_x.tile([KB, NKB, T], BF16, tag="xv")
        x_q = p_x.tile([KB, NKB, T], BF16, tag="xq")
        w_k = p_const.tile([KB, NKB, HALF], BF16, tag="wk")
        nc.sync.dma_start(w_k[:], wkT.rearrange("(kb p) j -> p kb j", p=KB))
        b_k = p_const.tile([KB, NJB], F32, tag="bk")
        nc.sync.dma_start(b_k[:], bk.rearrange("(jb p) one -> p (jb one)", p=KB))
        for kb in range(NKB):
            nc.sync.dma_start(x_k[:, kb, :], xkT[kb * KB : (kb + 1) * KB, :])
        w_v = p_const.tile([KB, NKB, HALF], BF16, tag="wv")
        nc.sync.dma_start(w_v[:], wvT.rearrange("(kb p) j -> p kb j", p=KB))
        for kb in range(NKB):
            nc.sync.dma_start(x_v[:, kb, :], xvT[kb * KB : (kb + 1) * KB, :])
        w_q = p_const.tile([KB, NKB, HALF], BF16, tag="wq")
        nc.sync.dma_start(w_q[:], wqT.rearrange("(kb p) j -> p kb j", p=KB))
        b_q = p_const.tile([KB, NJB], F32, tag="bq")
        nc.sync.dma_start(b_q[:], bq.rearrange("(jb p) one -> p (jb one)", p=KB))
        for kb in range(NKB):
            nc.sync.dma_start(x_q[:, kb, :], xqT[kb * KB : (kb + 1) * KB, :])
        w_o = p_const.tile([KB, NJB, D], BF16, tag="wo")
        nc.sync.dma_start(w_o[:], woT.rearrange("(jb p) n -> p jb n", p=KB))

        # ---- V tiles: [128 keys, 8 heads, DH+1] with a ones column ----
        v_tiles = [
            p_v.tile([KB, H // 2, DH + 1], BF16, tag="v", name=f"v{t}")
            for t in range(NTK)
        ]
        for t in range(NTK):
            nc.vector.memset(v_tiles[t][:, :, DH : DH + 1], 1.0)

        kt_tiles = [
            p_kt.tile([KB, T], BF16, tag="kt", name=f"kt{j}") for j in range(NJB)
        ]
        qt_tiles = [
            p_qt.tile([KB, T], BF16, tag="qt", name=f"qt{j}") for j in range(NJB)
        ]

        def kq_proj(w_t, x_t, b_t, out_tiles, tb, half):
            ps = p_sc.tile([KB, CH * TB], F32, tag="sc", name="pskq")
            for kb in range(NKB):
                for j2 in range(2):
                    jb = 2 * half + j2
                    nc.tensor.matmul(
                        ps[:, j2 * TB : (j2 + 1) * TB],
                        w_t[:, kb, jb * KB : (jb + 1) * KB],
                        x_t[:, kb, tb * TB : (tb + 1) * TB],
                        start=(kb == 0),
                        stop=(kb == NKB - 1),
                    )
            for j2 in range(2):
                jb = 2 * half + j2
                nc.vector.tensor_scalar_add(
                    out_tiles[jb][:, tb * TB : (tb + 1) * TB],
                    ps[:, j2 * TB : (j2 + 1) * TB],
                    b_t[:, jb : jb + 1],
                )

        # ---- K^T projection: kt[jb] is [128 (j), T] ----
        for tb in range(NTB):
            for half in range(2):
                kq_proj(w_k, x_k, b_k, kt_tiles, tb, half)

        # ---- V projection (natural layout) ----
        for tb in range(NTB):
            for half in range(2):
                ps = p_sc.tile([KB, CH * TB], F32, tag="sc", name="psv")
                for kb in range(NKB):
                    for t2 in range(2):
                        ts = 2 * half + t2
                        nc.tensor.matmul(
                            ps[:, t2 * TB : (t2 + 1) * TB],
                            x_v[:, kb, tb * TB + ts * KB : tb * TB + (ts + 1) * KB],
                            w_v[:, kb, :],
                            start=(kb == 0),
                            stop=(kb == NKB - 1),
                        )
                for t2 in range(2):
                    ts = 2 * half + t2
                    nc.vector.tensor_copy(
                        v_tiles[tb * 4 + ts][:, :, 0:DH],
                        ps[:, t2 * TB : (t2 + 1) * TB].rearrange(
                            "p (h d) -> p h d", d=DH
                        ),
                    )

        # ---- Q^T projection for tq=0 (later tq interleaved into attention) ----
        for half in range(2):
            kq_proj(w_q, x_q, b_q, qt_tiles, 0, half)

        # ---- deferred-work units interleaved into the attention stream ----
        def qproj_unit(tq1, jb):
            # one [128,512] q-projection block, borrowing a score slot
            def emit(state):
                ps = p_sc.tile([KB, TB], F32, tag="sc", name="psq")
                for kb in range(NKB):
                    nc.tensor.matmul(
                        ps[:],
                        w_q[:, kb, jb * KB : (jb + 1) * KB],
                        x_q[:, kb, tq1 * TB : (tq1 + 1) * TB],
                        start=(kb == 0),
                        stop=(kb == NKB - 1),
                    )
                nc.vector.tensor_scalar_add(
                    qt_tiles[jb][:, tq1 * TB : (tq1 + 1) * TB],
                    ps[:],
                    b_q[:, jb : jb + 1],
                )

            return emit

        def oproj_unit(tq0, ot_prev, nb, ts, on_act=False):
            def emit(state):
                po = p_sc.tile([KB, TB], F32, tag="sc", name="po")
                for jp in range(NJB):
                    nc.tensor.matmul(
                        po[:],
                        ot_prev[jp][:, ts * KB : (ts + 1) * KB],
                        w_o[:, jp, nb * TB : (nb + 1) * TB],
                        start=(jp == 0),
                        stop=(jp == NJB - 1),
                    )
                st = p_st.tile([KB, TB], BF16, tag="st", name="st")
                if on_act:
                    nc.scalar.copy(st[:], po[:])
                else:
                    nc.vector.tensor_copy(st[:], po[:])
                nc.sync.dma_start(
                    partial[
                        tq0 * TB + ts * KB : tq0 * TB + (ts + 1) * KB,
                        nb * TB : (nb + 1) * TB,
                    ],
                    st[:],
                )

            return emit

        # ---- attention per t-block of queries, pipelined across tq ----
        # head i=1 streams first: its normalize chain (with the
        # partition-shift DMA) hides under head i=0's matmuls; i=0's
        # shorter chain sits at the jp boundary.
        slots = [(1, tk) for tk in range(NTK)] + [(0, tk) for tk in range(NTK)]
        chunks = [slots[s : s + CH] for s in range(0, len(slots), CH)]
        ot_hist = {}
        carry = [None]
        for tq in range(NTB):
            # deferred units spread over the whole tq: O-proj of tq-1
            # first, then Q-proj quarters of tq+1
            pend_all = []
            if tq == 0:
                for tq1 in (1, 2):
                    for jb in range(NJB):
                        pend_all.append(qproj_unit(tq1, jb))
            elif tq == 1:
                for jb in range(NJB):
                    pend_all.append(qproj_unit(3, jb))
            if tq > 0:
                prev = ot_hist[tq - 1]
                for nb in range(2):
                    for ts in range(4):
                        pend_all.append(oproj_unit(tq - 1, prev, nb, ts))

            ot_tiles = [
                p_ot.tile([KB, TB], BF16, tag="ot", name=f"ot{tq}_{j}")
                for j in range(NJB)
            ]
            ot_hist[tq] = ot_tiles

            def chain_a(avs):
                # softmax denominator sits in row DH (ones column of V):
                # stage it and broadcast across the 64 head-dim partitions
                dsb = p_sm.tile([DH + 1, TB], F32, tag="dsb", name="dsb")
                nc.vector.tensor_copy(dsb[DH : DH + 1, :], avs[DH : DH + 1, :])
                bc = p_sm.tile([DH, TB], F32, tag="bc", name="bc", bufs=1)
                nc.sync.dma_start(
                    bc[:], dsb[DH : DH + 1, None, :].broadcast_to([1, DH, TB])
                )
                return bc

            def chain_b(bc, avs, jp, i, ot_tiles):
                # emitted ~2 chunks after chain_a so the reciprocal does not
                # head-of-line block the DVE queue during the DMA flight
                rc = p_sm.tile([DH, TB], F32, tag="rc", name="rc", bufs=1)
                nc.vector.reciprocal_approx_fast(rc[:], bc[:])
                if i == 0:
                    nc.vector.tensor_mul(ot_tiles[jp][0:DH, :], avs[0:DH, :], rc[:])
                else:
                    # DVE can't shift partitions; stage then DMA to rows 64:128
                    stg = p_sm.tile([DH, TB], BF16, tag="stg", name="stg")
                    nc.vector.tensor_mul(stg[:], avs[0:DH, :], rc[:])
                    nc.sync.dma_start(ot_tiles[jp][DH : 2 * DH, :], stg[:])

            state = {}
            for jp in range(NJB):
                # av0 is allocated after chunk 1, once the previous jp's
                # deferred chain_b has read the old av0 tile (av0's first
                # write is chunk 5)
                avs = {1: p_ap.tile([DH + 1, TB], F32, tag="av", name="av1")}
                ch1_bc = None
                for c, chunk in enumerate(chunks):
                    w = len(chunk) * TB
                    sc = p_sc.tile([KB, CH * TB], F32, tag="sc", name="sc")
                    for s, (i, tk) in enumerate(chunk):
                        nc.tensor.matmul(
                            sc[:, s * TB : (s + 1) * TB],
                            kt_tiles[jp][i * DH : (i + 1) * DH, tk * KB : (tk + 1) * KB],
                            qt_tiles[jp][i * DH : (i + 1) * DH, tq * TB : (tq + 1) * TB],
                            start=True,
                            stop=True,
                        )
                    ex = p_ex.tile([KB, CH * TB], BF16, tag="ex", name="ex")
                    nc.scalar.activation(
                        ex[:, :w], sc[:, :w], mybir.ActivationFunctionType.Exp,
                        scale=0.125,
                    )
                    for s, (i, tk) in enumerate(chunk):
                        nc.tensor.matmul(
                            avs[i][:],
                            v_tiles[tk][:, 2 * jp + i, :],
                            ex[:, s * TB : (s + 1) * TB],
                            start=(tk == 0),
                            stop=(tk == NTK - 1),
                        )
                    if c == 1:
                        if carry[0] is not None:
                            chain_b(*carry[0])
                        carry[0] = None
                        avs[0] = p_ap.tile(
                            [DH + 1, TB], F32, tag="av", name="av0"
                        )
                    if (1, NTK - 1) in chunk:
                        # head i=1 finished: its chain hides under head 0
                        ch1_bc = chain_a(avs[1])
                    if c == 7:
                        chain_b(ch1_bc, avs[1], jp, 1, ot_tiles)
                    if c in (3, 6, 9) and pend_all:
                        pend_all.pop(0)(state)
                carry[0] = (chain_a(avs[0]), avs[0], jp, 0, ot_tiles)
            for u in pend_all:
                u(state)

        # flush the last jp's deferred chain, then the out-projection for
        # the last t-block (psum->sbuf copies on the activation engine,
        # which is idle after the final exp). The first two tiles
        # accumulate jp 0-2 before the jp3 chain resolves.
        chain_b(*carry[0])
        carry[0] = None
        otl = ot_hist[NTB - 1]
        pos = {}
        for u, (nb, ts) in enumerate([(0, 0), (0, 1)]):
            pos[u] = p_sc.tile([KB, TB], F32, tag="sc", name="po")
            for jp in range(NJB - 1):
                nc.tensor.matmul(
                    pos[u][:],
                    otl[jp][:, ts * KB : (ts + 1) * KB],
                    w_o[:, jp, nb * TB : (nb + 1) * TB],
                    start=(jp == 0),
                    stop=False,
                )
        for u, (nb, ts) in enumerate([(0, 0), (0, 1)]):
            nc.tensor.matmul(
                pos[u][:],
                otl[NJB - 1][:, ts * KB : (ts + 1) * KB],
                w_o[:, NJB - 1, nb * TB : (nb + 1) * TB],
                start=False,
                stop=True,
            )
            st = p_st.tile([KB, TB], BF16, tag="st", name="st")
            nc.scalar.copy(st[:], pos[u][:])
            nc.sync.dma_start(
                partial[ts * KB + (NTB - 1) * TB : (ts + 1) * KB + (NTB - 1) * TB,
                        nb * TB : (nb + 1) * TB],
                st[:],
            )
        for nb, ts in [(0, 2), (0, 3), (1, 0), (1, 1), (1, 2), (1, 3)]:
            oproj_unit(NTB - 1, otl, nb, ts, on_act=True)({})

    nc.compile()
    return nc


def kernel(**inputs: np.ndarray) -> np.ndarray:
    query = np.asarray(inputs["query"], dtype=np.float32)
    key = np.asarray(inputs["key"], dtype=np.float32)
    value = np.asarray(inputs["value"], dtype=np.float32)
    w_q = np.asarray(inputs["w_q"], dtype=np.float32)
    b_q = np.asarray(inputs["b_q"], dtype=np.float32)
    w_k = np.asarray(inputs["w_k"], dtype=np.float32)
    b_k = np.asarray(inputs["b_k"], dtype=np.float32)
    w_v = np.asarray(inputs["w_v"], dtype=np.float32)
    b_v = np.asarray(inputs["b_v"], dtype=np.float32)
    w_o = np.asarray(inputs["w_o"], dtype=np.float32)
    b_o = np.asarray(inputs["b_o"], dtype=np.float32)

    nc = build_kernel()

    bf = ml_dtypes.bfloat16
    in_maps = []
    for c in range(N_CORES):
        b = c // 2
        hh = c % 2
        sl = slice(hh * HALF, (hh + 1) * HALF)
        in_maps.append(
            {
                "xqT": np.ascontiguousarray(query[b].T.astype(bf)),
                "xkT": np.ascontiguousarray(key[b].T.astype(bf)),
                "xvT": np.ascontiguousarray(value[b].T.astype(bf)),
                "wqT": np.ascontiguousarray(w_q[sl, :].T.astype(bf)),
                "wkT": np.ascontiguousarray(w_k[sl, :].T.astype(bf)),
                "wvT": np.ascontiguousarray(w_v[sl, :].T.astype(bf)),
                "woT": np.ascontiguousarray(w_o[:, sl].T.astype(bf)),
                "bq": np.ascontiguousarray(b_q[sl].reshape(HALF, 1)),
                "bk": np.ascontiguousarray(b_k[sl].reshape(HALF, 1)),
            }
        )

    res = run_bass_kernel_spmd(nc, in_maps, core_ids=list(range(N_CORES)))

    const_row = (b_v[None, :] @ w_o.T + b_o[None, :]).astype(np.float32)
    out = np.empty((B, T, D), dtype=np.float32)
    for b in range(B):
        out[b] = res.results[2 * b]["partial"].astype(np.float32) + res.results[
            2 * b + 1
        ]["partial"].astype(np.float32)
        out[b] += const_row
    return out
